# revision 16
# baseline (speedup 1.0000x reference)
"""TP-8 Trainium2 Bass kernel for a LLaDA/Llama transformer block.

Design (v4 — per-batch-serial schedule, queue-disciplined):
 - norm1 runs on the host: the device receives pre-normalized xn in
   fp8e4m3 (pair-packed for DoubleRow) and x.T/8 in fp16 (xs16).
 - q/k/v and o projections run fp8e4m3 with perf_mode=DoubleRow.
 - The residual joins INSIDE the o-projection eviction (osb = ps*IO +
   x/8, a single DVE stt), so the AllReduce output is x_mid directly
   and no post-AR assemble pass exists.
 - Per-batch-serial phases shorten the AR critical path: the whole b1
   attention pipeline covers AR-b0, and the b0 MLP covers AR-b1.
 - Queue discipline (the in-order engine queues are the whole game):
   Pool queue carries ONLY the 4 AR triggers; AR-gated x_mid loads sit
   on the SP queue pinned AFTER the MLP weight loads they'd otherwise
   block; norm2's square-sum matmuls are emitted in groups interleaved
   between ffup chains so the PE never head-of-line blocks on them.
 - norm2: squares in fp8 pair layout (ACT), sum via fp8-DR matmul with
   a ones stationary (2x fewer, 2x faster ms matmuls; quantization of
   x^2 averages out over D=4096).
 - Attention runs as head PAIRS with a one-step lg/exp software
   pipeline so den/at matmuls never wait on the Scalar exp latency.
 - MLP stays fp16 (fp8 there exceeds the 2e-2 error budget; measured:
   qkv fp8 8.4e-3 ok, ff/up fp8 4.6e-2 / wout fp8 2.4e-2 too large).

Sharding (per sharding_hint): tensor-parallel over 8 cores - q/k/v/ff
sharded on the output-feature axis (4 heads / 1536 ff dims per core),
wo/w_out sharded on the contraction axis; o-projection partials (+x/8)
AllReduced on device (fp16), final projection partials summed on host.
"""

from contextlib import ExitStack

import numpy as np
import ml_dtypes

import concourse.mybir as mybir
import concourse.tile as tile
from concourse import bacc
from concourse.bass_utils import run_bass_kernel_spmd

F32 = mybir.dt.float32
F16 = mybir.dt.float16
F8 = mybir.dt.float8e4
AF = mybir.ActivationFunctionType
ALU = mybir.AluOpType
DR = mybir.MatmulPerfMode.DoubleRow

N_CORES = 8
P = 128
B, T, D, FF = 2, 1024, 4096, 12288
M = B * T            # 2048 tokens
H = 128              # head dim
HALF = 64
QC = D // N_CORES    # 512 per-core q/k/v features (4 heads)
NH = QC // H         # 4 heads per core
FC = FF // N_CORES   # 1536 per-core ff features
NKP = D // P         # 32 K-tiles over D (fp16 granularity)
KP8 = NKP // 2       # 16 fp8 DoubleRow K-pair tiles
NFT = FC // P        # 12 M-tiles over per-core FF
NDT = D // P         # 32 D-tiles
NST = T // P         # 8 sequence tiles per batch
NCH = T // 512       # 2 column chunks per batch
EPS = 1e-05

SW = 16.0            # fp8 weight pre-scale
SX = 8.0             # fp8 xn pre-scale
SA = 32.0            # fp8 attnf pre-scale
IQK = 1.0 / (SW * SX)
IO = 1.0 / (SA * SW)


def _interleave(*items):
    """Drive generators to completion round-robin; an item may be a
    (generator, weight) tuple to take `weight` steps per round."""
    live = [[it[0], it[1]] if isinstance(it, tuple) else [it, 1] for it in items]
    while live:
        nxt = []
        for p in live:
            g, w = p
            alive = True
            for _ in range(w):
                try:
                    next(g)
                except StopIteration:
                    alive = False
                    break
            if alive:
                nxt.append(p)
        live = nxt


def _build():
    nc = bacc.Bacc("TRN2", target_bir_lowering=False, num_devices=N_CORES)

    xn8 = nc.declare_dram_parameter("xn8", [KP8, P, 2, M], F8, isOutput=False)
    xs16 = nc.declare_dram_parameter("xs16", [D, M], F16, isOutput=False)
    css = nc.declare_dram_parameter("css", [2, P, M], F16, isOutput=False)
    wq8 = nc.declare_dram_parameter("wq8", [NH, P, KP8, 2, P], F8, isOutput=False)
    wk8 = nc.declare_dram_parameter("wk8", [NH, P, KP8, 2, P], F8, isOutput=False)
    wv8 = nc.declare_dram_parameter("wv8", [P, KP8, 2, QC], F8, isOutput=False)
    wo8 = nc.declare_dram_parameter("wo8", [2, P, 2, NDT, P], F8, isOutput=False)
    wf_t = nc.declare_dram_parameter("wf_t", [NFT, P, NKP, P], F16, isOutput=False)
    wu_t = nc.declare_dram_parameter("wu_t", [NFT, P, NKP, P], F16, isOutput=False)
    wout_t = nc.declare_dram_parameter("wout_t", [NDT, P, NFT, P], F16, isOutput=False)
    y = nc.declare_dram_parameter("y", [D, M], F32, isOutput=True)

    with tile.TileContext(nc) as tc:
        _emit(nc, tc, xn8, xs16, css, wq8, wk8, wv8, wo8, wf_t, wu_t, wout_t, y)
    nc.compile()
    return nc


def _emit(nc, tc, xn8, xs16, css, wq8, wk8, wv8, wo8, wf_t, wu_t, wout_t, y):
    with ExitStack() as top:
        dram_pool = top.enter_context(tc.tile_pool(name="dram", bufs=1, space="DRAM"))
        const = top.enter_context(tc.tile_pool(name="const", bufs=1))

        cc_in = [dram_pool.tile([D, T], F16, name=f"cc_in_{b}") for b in range(B)]
        cc_out = [
            [
                dram_pool.tile([D // 2, T], F16, addr_space="Shared",
                               name=f"cc_out_{b}_{k}")
                for k in range(2)
            ]
            for b in range(B)
        ]

        def xmid_rows(b, kp):
            return cc_out[b][kp // (NKP // 2)][(kp % (NKP // 2)) * P:
                                               (kp % (NKP // 2) + 1) * P, :]

        ones_h = const.tile([P, P], F16)
        nc.vector.memset(ones_h[:], 1.0)
        ones8 = const.tile([P, 2, P], F8)
        nc.vector.memset(ones8[:], 1.0)
        eps_sb = const.tile([P, 1], F32)
        nc.vector.memset(eps_sb[:], EPS)

        # --- long-lived left-side tiles ---
        es_glob = ExitStack()
        glob = es_glob.enter_context(tc.tile_pool(name="glob", bufs=1))
        bcast2 = [glob.tile([P, T], F16, name=f"bcast2_{b}") for b in range(B)]
        attnf = [glob.tile([P, NH, T], F8, name=f"attnf_{b}") for b in range(B)]

        es_xn = ExitStack()
        xn_sp = es_xn.enter_context(tc.tile_pool(name="xn_sp", bufs=1))
        cc_sb = xn_sp.tile([P, M], F16, name="cc_sb")
        ss_sb = xn_sp.tile([P, M], F16, name="ss_sb")
        nc.sync.dma_start(out=cc_sb[:], in_=css[0])
        nc.sync.dma_start(out=ss_sb[:], in_=css[1])
        xn8k = []
        wv_sb = xn_sp.tile([P, KP8, 2, QC], F8, name="wv_sb")
        for kp in range(KP8):
            xn8k.append(xn_sp.tile([P, 2, M], F8, name=f"xn8_{kp}"))
        # first two xn tiles land before the first chain weights
        nc.sync.dma_start(out=xn8k[0][:], in_=xn8[0])
        nc.sync.dma_start(out=xn8k[1][:], in_=xn8[1])

        def gen_loader():
            """Streams the rest of the inputs between chain emissions so
            the first matmuls start ~10us in instead of ~45us."""
            for kp in range(2, KP8):
                nc.sync.dma_start(out=xn8k[kp][:], in_=xn8[kp])
                if kp == 9:
                    nc.sync.dma_start(out=wv_sb[:], in_=wv8[:])
                yield

        es_o_w = ExitStack()
        ow_sp = es_o_w.enter_context(tc.tile_pool(name="ow_sp", bufs=1))
        wo_sb = []

        # ============ attention superphase per batch ============
        def run_batch_attn(b, loader=None):
            with ExitStack() as es_a:
                qk_sp = es_a.enter_context(
                    tc.tile_pool(name=f"qk_s{b}", bufs=1, side="right")
                )
                bo_sp = es_a.enter_context(
                    tc.tile_pool(name=f"bo_s{b}", bufs=1, side="right")
                )
                es_qp = ExitStack()
                qk_pp = es_qp.enter_context(
                    tc.tile_pool(name=f"qk_p{b}", bufs=1, space="PSUM")
                )
                es_vp = ExitStack()
                v_pp = es_vp.enter_context(
                    tc.tile_pool(name=f"v_p{b}", bufs=1, space="PSUM",
                                 side="right")
                )

                qf = []
                kf = []
                v_sb = [None] * NST

                def gen_qk_chain(which, wsrc, dst, m):
                    wt = qk_sp.tile([P, KP8, 2, P], F8, tag="wqk", bufs=3,
                                    name=f"w{which}_{b}_{m}")
                    nc.sync.dma_start(out=wt[:], in_=wsrc[m])
                    out = bo_sp.tile([P, T], F16, name=f"{which}f_{b}_{m}")
                    for ch in range(NCH):
                        cs = slice(ch * 512, (ch + 1) * 512)
                        gcs = slice(b * T + ch * 512, b * T + (ch + 1) * 512)
                        ps = qk_pp.tile([P, 512], F32, tag="qk", bufs=3,
                                        name=f"ps{which}_{b}_{m}_{ch}")
                        for kp in range(KP8):
                            nc.tensor.matmul(
                                ps[:], wt[:, kp], xn8k[kp][:, :, gcs],
                                start=(kp == 0), stop=(kp == KP8 - 1),
                                perf_mode=DR,
                            )
                            if kp % 4 == 3:
                                yield
                        main = qk_sp.tile([P, 512], F16, tag="rmain", bufs=2,
                                          name=f"rm_{which}_{b}_{m}_{ch}")
                        nc.vector.scalar_tensor_tensor(
                            main[:], ps[:], IQK, cc_sb[:, gcs],
                            ALU.mult, ALU.mult,
                        )
                        rot = qk_sp.tile([P, 512], F16, tag="rrot", bufs=2,
                                         name=f"rr_{which}_{b}_{m}_{ch}")
                        nc.vector.scalar_tensor_tensor(
                            rot[:HALF], ps[HALF:], -IQK, ss_sb[:HALF, gcs],
                            ALU.mult, ALU.mult,
                        )
                        nc.vector.scalar_tensor_tensor(
                            rot[HALF:], ps[:HALF], IQK, ss_sb[HALF:, gcs],
                            ALU.mult, ALU.mult,
                        )
                        nc.vector.tensor_add(out[:, cs], main[:], rot[:])
                        yield
                    dst.append(out)

                def gen_v():
                    for r in range(NST // 2):
                        sts = (2 * r, 2 * r + 1)
                        psv = {}
                        for st in sts:
                            psv[st] = v_pp.tile([P, QC], F32, tag="vps",
                                                bufs=2, name=f"psv_{b}_{st}")
                        for kp in range(KP8):
                            for st in sts:
                                t0 = b * T + st * P
                                nc.tensor.matmul(
                                    psv[st][:], xn8k[kp][:, :, t0:t0 + P],
                                    wv_sb[:, kp],
                                    start=(kp == 0), stop=(kp == KP8 - 1),
                                    perf_mode=DR,
                                )
                            if kp % 4 == 3:
                                yield
                        for st in sts:
                            vt = bo_sp.tile([P, QC], F16, name=f"v_{b}_{st}")
                            nc.scalar.activation(vt[:], psv[st][:], AF.Copy,
                                                 scale=IQK)
                            v_sb[st] = vt
                        yield

                def gen_attn_head(att_pp, h):
                    """One-step lg/exp software pipeline; caller alternates
                    two heads so PE never waits on the exp latency."""
                    for qch in range(NCH):
                        qcs = slice(qch * 512, (qch + 1) * 512)
                        den_ps = att_pp.tile([P, 512], F32, tag="den", bufs=2,
                                             name=f"den_{b}_{h}_{qch}")
                        at_ps = att_pp.tile([P, 512], F32, tag="at", bufs=2,
                                            name=f"at_{b}_{h}_{qch}")

                        def emit_lgexp(st):
                            lg = att_pp.tile([P, 512], F32, tag="lg", bufs=2,
                                             name=f"lg_{b}_{h}_{qch}_{st}")
                            nc.tensor.matmul(
                                lg[:], kf[h][:, st * P:(st + 1) * P],
                                qf[h][:, qcs], start=True, stop=True,
                            )
                            pr = qk_sp.tile([P, 512], F16, tag="pr", bufs=4,
                                            name=f"pr_{b}_{h}_{qch}_{st}")
                            nc.scalar.activation(pr[:], lg[:], AF.Exp)
                            return pr

                        prs = [None] * NST
                        prs[0] = emit_lgexp(0)
                        yield
                        for st in range(NST):
                            if st + 1 < NST:
                                prs[st + 1] = emit_lgexp(st + 1)
                            yield
                            pr = prs[st]
                            nc.tensor.matmul(
                                den_ps[:], ones_h[:], pr[:],
                                start=(st == 0), stop=(st == NST - 1),
                            )
                            nc.tensor.matmul(
                                at_ps[:], v_sb[st][:, h * H:(h + 1) * H],
                                pr[:],
                                start=(st == 0), stop=(st == NST - 1),
                            )
                            yield
                        rec = qk_sp.tile([P, 512], F32, tag="rec", bufs=2,
                                         name=f"rec_{b}_{h}_{qch}")
                        nc.vector.reciprocal_approx_fast(rec[:], den_ps[:])
                        nc.vector.scalar_tensor_tensor(
                            attnf[b][:, h, qcs], at_ps[:], SA, rec[:],
                            ALU.mult, ALU.mult,
                        )
                        yield

                qg = [gen_qk_chain("q", wq8, qf, m) for m in range(NH)]
                kg = [gen_qk_chain("k", wk8, kf, m) for m in range(NH)]
                vg = gen_v()
                if loader is not None:
                    # loader FIRST and fast enough that every xn8k DMA is
                    # emitted before the first chain matmul that reads it
                    # (the dependency tracker is emission-order based)
                    _interleave((loader, 4), qg[0], kg[0])
                    _interleave((loader, 4), qg[1], kg[1])
                    _interleave(loader)
                else:
                    _interleave(qg[0], kg[0])
                    _interleave(qg[1], kg[1])
                _interleave(qg[2], kg[2], (vg, 2))
                _interleave(qg[3], kg[3], (vg, 2))
                _interleave(vg)
                es_vp.close()
                es_qp.close()
                if b == 0:
                    for hp in range(2):
                        wt = ow_sp.tile([P, 2, NDT, P], F8, name=f"wo_sb_{hp}")
                        nc.sync.dma_start(out=wt[:], in_=wo8[hp])
                        wo_sb.append(wt)
                # PSUM: qk 2 + den 2 + at 2 + lg 2 = 8 banks
                es_ap = ExitStack()
                att_pp = es_ap.enter_context(
                    tc.tile_pool(name=f"att_p{b}", bufs=1, space="PSUM")
                )
                _interleave(gen_attn_head(att_pp, 0), gen_attn_head(att_pp, 1))
                _interleave(gen_attn_head(att_pp, 2), gen_attn_head(att_pp, 3))
                es_ap.close()

        # ============ o-projection + AR per batch ============
        def fire_ar(b, k):
            rows = slice(k * (D // 2), (k + 1) * (D // 2))
            nc.gpsimd.collective_compute(
                "AllReduce",
                ALU.add,
                replica_groups=[list(range(N_CORES))],
                ins=[cc_in[b][rows, :]],
                outs=[cc_out[b][k][:, :]],
            )

        def run_o_proj(b):
            """o-projection partials + x/8 residual; the eviction stream is
            split DVE / (ACT copy + Pool add) per chunk so no single engine
            paces the PE, and the AR triggers fire as early as possible."""
            with ExitStack() as es_o:
                o_sp = es_o.enter_context(
                    tc.tile_pool(name=f"o_s{b}", bufs=1, side="right")
                )
                o_pp = es_o.enter_context(
                    tc.tile_pool(name=f"o_p{b}", bufs=1, space="PSUM",
                                 side="right")
                )
                for dt in range(NDT):
                    for ch in range(NCH):
                        cs = slice(ch * 512, (ch + 1) * 512)
                        gcs = slice(b * T + ch * 512, b * T + (ch + 1) * 512)
                        xt = o_sp.tile([P, 512], F16, tag="xres", bufs=6,
                                       name=f"xr_{b}_{dt}_{ch}")
                        nc.sync.dma_start(
                            out=xt[:], in_=xs16[dt * P:(dt + 1) * P, gcs]
                        )
                        ps = o_pp.tile([P, 512], F32, tag="o", bufs=4,
                                       name=f"pso_{b}_{dt}_{ch}")
                        for hp in range(2):
                            nc.tensor.matmul(
                                ps[:], wo_sb[hp][:, :, dt, :],
                                attnf[b][:, 2 * hp:2 * hp + 2, cs],
                                start=(hp == 0), stop=(hp == 1), perf_mode=DR,
                            )
                        osb = o_sp.tile([P, 512], F16, tag="osb", bufs=6,
                                        name=f"osb_{b}_{dt}_{ch}")
                        if dt % 2 == 0:
                            nc.vector.scalar_tensor_tensor(
                                osb[:], ps[:], IO, xt[:], ALU.mult, ALU.add
                            )
                        else:
                            tmp = o_sp.tile([P, 512], F16, tag="otmp",
                                            bufs=4, name=f"ot_{b}_{dt}_{ch}")
                            nc.scalar.activation(tmp[:], ps[:], AF.Copy,
                                                 scale=IO)
                            nc.gpsimd.tensor_add(osb[:], tmp[:], xt[:])
                        nc.sync.dma_start(
                            out=cc_in[b][dt * P:(dt + 1) * P, cs], in_=osb[:]
                        )
                    if dt == NDT // 2 - 1 or dt == NDT - 1:
                        fire_ar(b, 0 if dt < NDT // 2 else 1)

        # ---------- phases A(0), O(0), A(1), O(1) ----------
        ld = gen_loader()
        run_batch_attn(0, loader=ld)
        run_o_proj(0)
        run_batch_attn(1)
        # pinned past the xmid-b0 loads (2.15) so o-b1's ACT copies sort
        # after them in the Scalar queue; real timing is semaphore-driven
        with tc.tile_wait_until(2.18):
            run_o_proj(1)

        es_o_w.close()
        es_xn.close()

        # ============ MLP phases (pinned after the ARs) ============
        def load_xmid(b, pool, xdst):
            # issued from the Scalar DGE so these AR-gated loads never block
            # the SP DMA queue that feeds weight/eviction traffic
            for kp in range(NKP):
                xk = pool.tile([P, T], F16, name=f"xm{b}_{kp}")
                nc.scalar.dma_start(out=xk[:], in_=xmid_rows(b, kp))
                xdst.append(xk)

        def mlp_ffup(b, xmh, w_sp, h_sp, ff_pp, ms_pp, hsb):
            """norm2 (fp8 squares + DR ones-matmul) interleaved with the
            ff/up chains: ms groups are emitted between chains so the PE
            reaches them only after their squares exist, and bcast2 is
            ready before the first eviction's WAR deadline (ps bufs=3)."""
            ms_ps = ms_pp.tile([P, T], F32, tag="ms", bufs=1, name=f"ms_{b}")
            sq8 = []

            def emit_sq(j):  # squares for kp pair j -> fp8 pair tile
                sq = w_sp.tile([P, 2, T], F8, tag="sq", bufs=NKP // 2,
                               name=f"sq_{b}_{j}")
                for e in range(2):
                    nc.scalar.activation(sq[:, e, :], xmh[2 * j + e][:],
                                         AF.Square)
                sq8.append(sq)

            def emit_ms_group(g):  # 4 kp-pairs of ms accumulation
                for j in range(4 * g, 4 * g + 4):
                    for ch in range(NCH):
                        cs = slice(ch * 512, (ch + 1) * 512)
                        nc.tensor.matmul(
                            ms_ps[:, cs], ones8[:], sq8[j][:, :, cs],
                            start=(j == 0), stop=(j == KP8 - 1),
                            perf_mode=DR,
                        )

            def finish_norm2():
                lnt = w_sp.tile([P, T], F32, tag="lnt", bufs=1,
                                name=f"lnt_{b}")
                nc.scalar.activation(lnt[:], ms_ps[:], AF.Ln, bias=eps_sb[:],
                                     scale=1.0 / D)
                nc.scalar.activation(bcast2[b][:], lnt[:], AF.Exp, scale=-0.5)

            ffs = [None] * NFT

            def emit_chain_mm(m, which, wsrc):
                wt = w_sp.tile([P, NKP, P], F16, tag="wffu", bufs=3,
                               name=f"w{which}_{b}_{m}")
                nc.sync.dma_start(out=wt[:], in_=wsrc[m])
                pss = []
                for ch in range(NCH):
                    cs = slice(ch * 512, (ch + 1) * 512)
                    ps = ff_pp.tile([P, 512], F32, tag=f"ps_{which}", bufs=3,
                                    name=f"ps{which}_{b}_{m}_{ch}")
                    for kp in range(NKP):
                        nc.tensor.matmul(
                            ps[:], wt[:, kp, :], xmh[kp][:, cs],
                            start=(kp == 0), stop=(kp == NKP - 1),
                        )
                    pss.append(ps)
                return pss

            def emit_evict(m, which, pss):
                # reads bcast2 — must be emitted AFTER finish_norm2 so the
                # emission-order dependency tracker sees the write
                for ch in range(NCH):
                    cs = slice(ch * 512, (ch + 1) * 512)
                    nt = w_sp.tile([P, 512], F16, tag=f"nrm_{which}", bufs=3,
                                   name=f"nt{which}_{b}_{m}_{ch}")
                    nc.vector.scalar_tensor_tensor(
                        nt[:], pss[ch][:], 1.0, bcast2[b][:, cs],
                        ALU.mult, ALU.mult,
                    )
                    if which == "f":
                        nc.scalar.activation(ffs[m][:, cs], nt[:], AF.Silu)
                    else:
                        nc.vector.tensor_mul(hsb[m][:, cs], nt[:],
                                             ffs[m][:, cs])

            def emit_chain(m, which, wsrc):
                emit_evict(m, which, emit_chain_mm(m, which, wsrc))

            for j in range(KP8):
                emit_sq(j)
            for m in range(NFT):
                ffs[m] = w_sp.tile([P, T], F16, tag="ffs", bufs=3,
                                   name=f"ff_{b}_{m}")
                hsb.append(h_sp.tile([P, T], F16, tag=f"h{m}",
                                     name=f"h_{b}_{m}"))
            psf0 = emit_chain_mm(0, "f", wf_t)
            emit_ms_group(0)
            psu0 = emit_chain_mm(0, "u", wu_t)
            emit_ms_group(1)
            emit_ms_group(2)
            emit_ms_group(3)
            finish_norm2()
            emit_evict(0, "f", psf0)
            emit_evict(0, "u", psu0)
            for m in range(1, NFT):
                emit_chain(m, "f", wf_t)
                emit_chain(m, "u", wu_t)

        def emit_wout(b, xmh, hsb, wo2_sp, wo2_pp):
            for dt in range(NDT):
                wt = wo2_sp.tile([P, NFT, P], F16, tag="wot", bufs=2,
                                 name=f"wot_{b}_{dt}")
                nc.sync.dma_start(out=wt[:], in_=wout_t[dt])
                for ch in range(NCH):
                    cs = slice(ch * 512, (ch + 1) * 512)
                    ps = wo2_pp.tile([P, 512], F32, tag="o2", bufs=2,
                                     name=f"pso2_{b}_{dt}_{ch}")
                    for mm in range(NFT):
                        nc.tensor.matmul(
                            ps[:], wt[:, mm, :], hsb[mm][:, cs],
                            start=(mm == 0), stop=(mm == NFT - 1),
                        )
                    ysb = wo2_sp.tile([P, 512], F32, tag="ysb", bufs=2,
                                      name=f"ysb_{b}_{dt}_{ch}")
                    nc.vector.scalar_tensor_tensor(
                        ysb[:], xmh[dt][:, cs], 1.0 / N_CORES, ps[:],
                        ALU.mult, ALU.add,
                    )
                    nc.sync.dma_start(
                        out=y[dt * P:(dt + 1) * P,
                              b * T + ch * 512:b * T + (ch + 1) * 512],
                        in_=ysb[:],
                    )

        # P7: x_mid b0 loads — Scalar queue, between A(1)'s exps and
        # o-b1's eviction copies (2.15 < 2.18)
        es_x0 = ExitStack()
        x0_sp = es_x0.enter_context(tc.tile_pool(name="x0_sp", bufs=1))
        xmh0 = []
        with tc.tile_wait_until(2.15):
            load_xmid(0, x0_sp, xmh0)

        # P8: norm2 b0 + ffup b0 (PSUM: ms 2 + ps_f 3 + ps_u 3 = 8 banks)
        hsb0 = []
        with tc.tile_wait_until(2.2):
            with ExitStack() as es_f0:
                f0w_sp = es_f0.enter_context(
                    tc.tile_pool(name="f0w_sp", bufs=1, side="right")
                )
                ff0_pp = es_f0.enter_context(
                    tc.tile_pool(name="f0_p", bufs=1, space="PSUM")
                )
                ms0_pp = es_f0.enter_context(
                    tc.tile_pool(name="ms0_p", bufs=1, space="PSUM",
                                 side="right")
                )
                mlp_ffup(0, xmh0, f0w_sp, x0_sp, ff0_pp, ms0_pp, hsb0)

        # P9: wout b0
        with tc.tile_wait_until(2.6):
            with ExitStack() as es_w0:
                w0_sp = es_w0.enter_context(
                    tc.tile_pool(name="w0_sp", bufs=1, side="right")
                )
                w0_pp = es_w0.enter_context(
                    tc.tile_pool(name="w0_p", bufs=1, space="PSUM")
                )
                emit_wout(0, xmh0, hsb0, w0_sp, w0_pp)

        # P9.5: x_mid b1 loads (SP, after wout-b0 weights)
        es_x1 = ExitStack()
        x1_sp = es_x1.enter_context(tc.tile_pool(name="x1_sp", bufs=1,
                                                 side="right"))
        xmh1 = []
        with tc.tile_wait_until(2.65):
            load_xmid(1, x1_sp, xmh1)
        es_x0.close()

        # P10: norm2 b1 + ffup b1
        hsb1 = []
        with tc.tile_wait_until(2.7):
            with ExitStack() as es_f1:
                f1w_sp = es_f1.enter_context(
                    tc.tile_pool(name="f1w_sp", bufs=1)
                )
                ff1_pp = es_f1.enter_context(
                    tc.tile_pool(name="f1_p", bufs=1, space="PSUM")
                )
                ms1_pp = es_f1.enter_context(
                    tc.tile_pool(name="ms1_p", bufs=1, space="PSUM",
                                 side="right")
                )
                mlp_ffup(1, xmh1, f1w_sp, x1_sp, ff1_pp, ms1_pp, hsb1)

        # P11: wout b1
        with tc.tile_wait_until(3.0):
            with ExitStack() as es_w1:
                w1_sp = es_w1.enter_context(
                    tc.tile_pool(name="w1_sp", bufs=1)
                )
                w1_pp = es_w1.enter_context(
                    tc.tile_pool(name="w1_p", bufs=1, space="PSUM")
                )
                emit_wout(1, xmh1, hsb1, w1_sp, w1_pp)
        es_x1.close()
        es_glob.close()


_NC_CACHE = {}


def _get_nc():
    if "nc" not in _NC_CACHE:
        _NC_CACHE["nc"] = _build()
    return _NC_CACHE["nc"]


def _host_prep(x, sin, cos, attn_norm_w, ff_norm_w, wq, wk, wv, wo, w_ff, w_up, w_out):
    f16 = np.float16
    f8 = ml_dtypes.float8_e4m3
    x2 = np.asarray(x, np.float32).reshape(M, D)
    xT = np.ascontiguousarray(x2.T)  # [D, M]

    # host norm1: per-token rms scale folded into a pre-normalized xn
    rs1 = 1.0 / np.sqrt((x2 * x2).mean(-1) + EPS)  # [M]
    xn = xT * rs1[None, :]
    # fp8 pair-packed [kp, p, e, t]: contraction k = kp*256 + e*128 + p
    xn8 = np.ascontiguousarray(
        (xn * SX).astype(f8).reshape(KP8, 2, P, M).transpose(0, 2, 1, 3)
    )

    sinT = np.asarray(sin, np.float32).reshape(M, HALF).T
    cosT = np.asarray(cos, np.float32).reshape(M, HALF).T
    cc = np.concatenate([cosT, cosT], axis=0)
    ss = np.concatenate([sinT, sinT], axis=0)
    css = np.stack([cc, ss]).astype(f16)

    anw = np.asarray(attn_norm_w, np.float32)[:, None]
    fnw = np.asarray(ff_norm_w, np.float32)[:, None]
    wqn = (anw * np.asarray(wq, np.float32)) * (H ** -0.5)
    wkn = anw * np.asarray(wk, np.float32)
    wvn = anw * np.asarray(wv, np.float32)
    wfn = fnw * np.asarray(w_ff, np.float32)
    wun = fnw * np.asarray(w_up, np.float32)
    wo_f = np.asarray(wo, np.float32)
    w_out_f = np.asarray(w_out, np.float32)
    # x/8 residual: every core adds this in the o-proj eviction, so the
    # AllReduce over 8 cores reconstructs x exactly once.
    xs16 = (xT * (1.0 / N_CORES)).astype(f16)

    def pack_qk(w):  # [D, QC] -> [NH, P, KP8, 2, P] fp8, scaled
        return np.ascontiguousarray(
            (w * SW).astype(f8).reshape(KP8, 2, P, NH, P).transpose(3, 2, 0, 1, 4)
        )

    def mtile(w):
        # [K, F] -> [F/P, P, K/P, P] with [m, p, kp, j] = w[kp*P+p, m*P+j]
        K, F = w.shape
        return np.ascontiguousarray(
            w.reshape(K // P, P, F // P, P).transpose(2, 1, 0, 3)
        )

    in_maps = []
    for c in range(N_CORES):
        qs = slice(c * QC, (c + 1) * QC)
        fs = slice(c * FC, (c + 1) * FC)
        wv8 = np.ascontiguousarray(
            (wvn[:, qs] * SW).astype(f8).reshape(KP8, 2, P, QC).transpose(2, 0, 1, 3)
        )
        wo8 = np.ascontiguousarray(
            (wo_f[qs, :] * SW).astype(f8).reshape(2, 2, P, NDT, P)
            .transpose(0, 2, 1, 3, 4)
        )
        in_maps.append(
            {
                "xn8": xn8,
                "xs16": xs16,
                "css": css,
                "wq8": pack_qk(wqn[:, qs]),
                "wk8": pack_qk(wkn[:, qs]),
                "wv8": wv8,
                "wo8": wo8,
                "wf_t": mtile(wfn[:, fs]).astype(f16),
                "wu_t": mtile(wun[:, fs]).astype(f16),
                "wout_t": mtile(w_out_f[fs, :]).astype(f16),
            }
        )
    return in_maps


def kernel(**inputs) -> np.ndarray:
    nc = _get_nc()
    in_maps = _host_prep(**inputs)
    res = run_bass_kernel_spmd(
        nc, in_maps, core_ids=list(range(N_CORES)), trace=False
    )
    acc = res.results[0]["y"].astype(np.float64)
    for c in range(1, N_CORES):
        acc += res.results[c]["y"]
    return np.ascontiguousarray(acc.T).astype(np.float32).reshape(B, T, D)


# revision 20
# speedup vs baseline: 1.0237x; 1.0237x over previous
"""TP-8 Trainium2 Bass kernel for a LLaDA/Llama transformer block.

Design (v4 — per-batch-serial schedule, queue-disciplined):
 - norm1 runs on the host: the device receives pre-normalized xn in
   fp8e4m3 (pair-packed for DoubleRow) and x.T/8 in fp16 (xs16).
 - q/k/v and o projections run fp8e4m3 with perf_mode=DoubleRow.
 - The residual joins INSIDE the o-projection eviction (osb = ps*IO +
   x/8, a single DVE stt), so the AllReduce output is x_mid directly
   and no post-AR assemble pass exists.
 - Per-batch-serial phases shorten the AR critical path: the whole b1
   attention pipeline covers AR-b0, and the b0 MLP covers AR-b1.
 - Queue discipline (the in-order engine queues are the whole game):
   Pool queue carries ONLY the 4 AR triggers; AR-gated x_mid loads sit
   on the SP queue pinned AFTER the MLP weight loads they'd otherwise
   block; norm2's square-sum matmuls are emitted in groups interleaved
   between ffup chains so the PE never head-of-line blocks on them.
 - norm2: squares in fp8 pair layout (ACT), sum via fp8-DR matmul with
   a ones stationary (2x fewer, 2x faster ms matmuls; quantization of
   x^2 averages out over D=4096).
 - Attention runs as head PAIRS with a one-step lg/exp software
   pipeline so den/at matmuls never wait on the Scalar exp latency.
 - MLP stays fp16 (fp8 there exceeds the 2e-2 error budget; measured:
   qkv fp8 8.4e-3 ok, ff/up fp8 4.6e-2 / wout fp8 2.4e-2 too large).

Sharding (per sharding_hint): tensor-parallel over 8 cores - q/k/v/ff
sharded on the output-feature axis (4 heads / 1536 ff dims per core),
wo/w_out sharded on the contraction axis; o-projection partials (+x/8)
AllReduced on device (fp16), final projection partials summed on host.
"""

from contextlib import ExitStack

import numpy as np
import ml_dtypes

import concourse.mybir as mybir
import concourse.tile as tile
from concourse import bacc
from concourse.bass_utils import run_bass_kernel_spmd

F32 = mybir.dt.float32
F16 = mybir.dt.float16
F8 = mybir.dt.float8e4
AF = mybir.ActivationFunctionType
ALU = mybir.AluOpType
DR = mybir.MatmulPerfMode.DoubleRow

N_CORES = 8
P = 128
B, T, D, FF = 2, 1024, 4096, 12288
M = B * T            # 2048 tokens
H = 128              # head dim
HALF = 64
QC = D // N_CORES    # 512 per-core q/k/v features (4 heads)
NH = QC // H         # 4 heads per core
FC = FF // N_CORES   # 1536 per-core ff features
NKP = D // P         # 32 K-tiles over D (fp16 granularity)
KP8 = NKP // 2       # 16 fp8 DoubleRow K-pair tiles
NFT = FC // P        # 12 M-tiles over per-core FF
NDT = D // P         # 32 D-tiles
NST = T // P         # 8 sequence tiles per batch
NCH = T // 512       # 2 column chunks per batch
EPS = 1e-05

SW = 16.0            # fp8 weight pre-scale
SX = 8.0             # fp8 xn pre-scale
SA = 32.0            # fp8 attnf pre-scale
IQK = 1.0 / (SW * SX)
IO = 1.0 / (SA * SW)


def _interleave(*items):
    """Drive generators to completion round-robin; an item may be a
    (generator, weight) tuple to take `weight` steps per round."""
    live = [[it[0], it[1]] if isinstance(it, tuple) else [it, 1] for it in items]
    while live:
        nxt = []
        for p in live:
            g, w = p
            alive = True
            for _ in range(w):
                try:
                    next(g)
                except StopIteration:
                    alive = False
                    break
            if alive:
                nxt.append(p)
        live = nxt


def _build():
    nc = bacc.Bacc("TRN2", target_bir_lowering=False, num_devices=N_CORES)

    xn8 = nc.declare_dram_parameter("xn8", [KP8, P, 2, M], F8, isOutput=False)
    xs16 = nc.declare_dram_parameter("xs16", [D, M], F16, isOutput=False)
    css = nc.declare_dram_parameter("css", [2, P, M], F16, isOutput=False)
    wq8 = nc.declare_dram_parameter("wq8", [NH, P, KP8, 2, P], F8, isOutput=False)
    wk8 = nc.declare_dram_parameter("wk8", [NH, P, KP8, 2, P], F8, isOutput=False)
    wv8 = nc.declare_dram_parameter("wv8", [P, KP8, 2, QC], F8, isOutput=False)
    wo8 = nc.declare_dram_parameter("wo8", [2, P, 2, NDT, P], F8, isOutput=False)
    wf_t = nc.declare_dram_parameter("wf_t", [NFT, P, NKP, P], F16, isOutput=False)
    wu_t = nc.declare_dram_parameter("wu_t", [NFT, P, NKP, P], F16, isOutput=False)
    wout_t = nc.declare_dram_parameter("wout_t", [NDT, P, NFT, P], F16, isOutput=False)
    y = nc.declare_dram_parameter("y", [D, M], F32, isOutput=True)

    with tile.TileContext(nc) as tc:
        _emit(nc, tc, xn8, xs16, css, wq8, wk8, wv8, wo8, wf_t, wu_t, wout_t, y)
    nc.compile()
    return nc


def _emit(nc, tc, xn8, xs16, css, wq8, wk8, wv8, wo8, wf_t, wu_t, wout_t, y):
    with ExitStack() as top:
        dram_pool = top.enter_context(tc.tile_pool(name="dram", bufs=1, space="DRAM"))
        const = top.enter_context(tc.tile_pool(name="const", bufs=1))

        cc_in = [dram_pool.tile([D, T], F16, name=f"cc_in_{b}") for b in range(B)]
        cc_out = [
            [
                dram_pool.tile([D // 2, T], F16, addr_space="Shared",
                               name=f"cc_out_{b}_{k}")
                for k in range(2)
            ]
            for b in range(B)
        ]

        def xmid_rows(b, kp):
            return cc_out[b][kp // (NKP // 2)][(kp % (NKP // 2)) * P:
                                               (kp % (NKP // 2) + 1) * P, :]

        ones_h = const.tile([P, P], F16)
        nc.vector.memset(ones_h[:], 1.0)
        ones8 = const.tile([P, 2, P], F8)
        nc.vector.memset(ones8[:], 1.0)
        eps_sb = const.tile([P, 1], F32)
        nc.vector.memset(eps_sb[:], EPS)

        # --- long-lived left-side tiles ---
        es_glob = ExitStack()
        glob = es_glob.enter_context(tc.tile_pool(name="glob", bufs=1))
        bcast2 = [glob.tile([P, T], F16, name=f"bcast2_{b}") for b in range(B)]
        attnf = [glob.tile([P, NH, T], F8, name=f"attnf_{b}") for b in range(B)]

        es_xn = ExitStack()
        xn_sp = es_xn.enter_context(tc.tile_pool(name="xn_sp", bufs=1))
        cc_sb = xn_sp.tile([P, M], F16, name="cc_sb")
        ss_sb = xn_sp.tile([P, M], F16, name="ss_sb")
        nc.sync.dma_start(out=cc_sb[:], in_=css[0])
        nc.sync.dma_start(out=ss_sb[:], in_=css[1])
        xn8k = []
        wv_sb = xn_sp.tile([P, KP8, 2, QC], F8, name="wv_sb")
        for kp in range(KP8):
            xn8k.append(xn_sp.tile([P, 2, M], F8, name=f"xn8_{kp}"))
        # first two xn tiles land before the first chain weights
        nc.sync.dma_start(out=xn8k[0][:], in_=xn8[0])
        nc.sync.dma_start(out=xn8k[1][:], in_=xn8[1])

        def gen_loader():
            """Streams the rest of the inputs between chain emissions so
            the first matmuls start ~10us in instead of ~45us."""
            for kp in range(2, KP8):
                nc.sync.dma_start(out=xn8k[kp][:], in_=xn8[kp])
                if kp == 9:
                    nc.sync.dma_start(out=wv_sb[:], in_=wv8[:])
                yield

        es_o_w = ExitStack()
        ow_sp = es_o_w.enter_context(tc.tile_pool(name="ow_sp", bufs=1))
        wo_sb = []

        # ============ attention superphase per batch ============
        def run_batch_attn(b, loader=None):
            with ExitStack() as es_a:
                qk_sp = es_a.enter_context(
                    tc.tile_pool(name=f"qk_s{b}", bufs=1, side="right")
                )
                bo_sp = es_a.enter_context(
                    tc.tile_pool(name=f"bo_s{b}", bufs=1, side="right")
                )
                es_qp = ExitStack()
                qk_pp = es_qp.enter_context(
                    tc.tile_pool(name=f"qk_p{b}", bufs=1, space="PSUM")
                )
                es_vp = ExitStack()
                v_pp = es_vp.enter_context(
                    tc.tile_pool(name=f"v_p{b}", bufs=1, space="PSUM",
                                 side="right")
                )

                qf = []
                kf = []
                v_sb = [None] * NST

                def gen_qk_chain(which, wsrc, dst, m):
                    wt = qk_sp.tile([P, KP8, 2, P], F8, tag="wqk", bufs=3,
                                    name=f"w{which}_{b}_{m}")
                    nc.sync.dma_start(out=wt[:], in_=wsrc[m])
                    out = bo_sp.tile([P, T], F16, name=f"{which}f_{b}_{m}")
                    for ch in range(NCH):
                        cs = slice(ch * 512, (ch + 1) * 512)
                        gcs = slice(b * T + ch * 512, b * T + (ch + 1) * 512)
                        ps = qk_pp.tile([P, 512], F32, tag="qk", bufs=3,
                                        name=f"ps{which}_{b}_{m}_{ch}")
                        for kp in range(KP8):
                            nc.tensor.matmul(
                                ps[:], wt[:, kp], xn8k[kp][:, :, gcs],
                                start=(kp == 0), stop=(kp == KP8 - 1),
                                perf_mode=DR,
                            )
                            if kp % 4 == 3:
                                yield
                        main = qk_sp.tile([P, 512], F16, tag="rmain", bufs=2,
                                          name=f"rm_{which}_{b}_{m}_{ch}")
                        nc.vector.scalar_tensor_tensor(
                            main[:], ps[:], IQK, cc_sb[:, gcs],
                            ALU.mult, ALU.mult,
                        )
                        rot = qk_sp.tile([P, 512], F16, tag="rrot", bufs=2,
                                         name=f"rr_{which}_{b}_{m}_{ch}")
                        nc.vector.scalar_tensor_tensor(
                            rot[:HALF], ps[HALF:], -IQK, ss_sb[:HALF, gcs],
                            ALU.mult, ALU.mult,
                        )
                        nc.vector.scalar_tensor_tensor(
                            rot[HALF:], ps[:HALF], IQK, ss_sb[HALF:, gcs],
                            ALU.mult, ALU.mult,
                        )
                        nc.vector.tensor_add(out[:, cs], main[:], rot[:])
                        yield
                    dst.append(out)

                def gen_v():
                    for r in range(NST // 2):
                        sts = (2 * r, 2 * r + 1)
                        psv = {}
                        for st in sts:
                            psv[st] = v_pp.tile([P, QC], F32, tag="vps",
                                                bufs=2, name=f"psv_{b}_{st}")
                        for kp in range(KP8):
                            for st in sts:
                                t0 = b * T + st * P
                                nc.tensor.matmul(
                                    psv[st][:], xn8k[kp][:, :, t0:t0 + P],
                                    wv_sb[:, kp],
                                    start=(kp == 0), stop=(kp == KP8 - 1),
                                    perf_mode=DR,
                                )
                            if kp % 4 == 3:
                                yield
                        for st in sts:
                            vt = bo_sp.tile([P, QC], F16, name=f"v_{b}_{st}")
                            nc.scalar.activation(vt[:], psv[st][:], AF.Copy,
                                                 scale=IQK)
                            v_sb[st] = vt
                        yield

                def gen_attn_head(att_pp, h):
                    """One-step lg/exp software pipeline; caller alternates
                    two heads so PE never waits on the exp latency."""
                    for qch in range(NCH):
                        qcs = slice(qch * 512, (qch + 1) * 512)
                        den_ps = att_pp.tile([P, 512], F32, tag="den", bufs=2,
                                             name=f"den_{b}_{h}_{qch}")
                        at_ps = att_pp.tile([P, 512], F32, tag="at", bufs=2,
                                            name=f"at_{b}_{h}_{qch}")

                        def emit_lgexp(st):
                            lg = att_pp.tile([P, 512], F32, tag="lg", bufs=2,
                                             name=f"lg_{b}_{h}_{qch}_{st}")
                            nc.tensor.matmul(
                                lg[:], kf[h][:, st * P:(st + 1) * P],
                                qf[h][:, qcs], start=True, stop=True,
                            )
                            pr = qk_sp.tile([P, 512], F16, tag="pr", bufs=4,
                                            name=f"pr_{b}_{h}_{qch}_{st}")
                            nc.scalar.activation(pr[:], lg[:], AF.Exp)
                            return pr

                        prs = [None] * NST
                        prs[0] = emit_lgexp(0)
                        yield
                        for st in range(NST):
                            if st + 1 < NST:
                                prs[st + 1] = emit_lgexp(st + 1)
                            yield
                            pr = prs[st]
                            nc.tensor.matmul(
                                den_ps[:], ones_h[:], pr[:],
                                start=(st == 0), stop=(st == NST - 1),
                            )
                            nc.tensor.matmul(
                                at_ps[:], v_sb[st][:, h * H:(h + 1) * H],
                                pr[:],
                                start=(st == 0), stop=(st == NST - 1),
                            )
                            yield
                        rec = qk_sp.tile([P, 512], F32, tag="rec", bufs=2,
                                         name=f"rec_{b}_{h}_{qch}")
                        nc.vector.reciprocal_approx_fast(rec[:], den_ps[:])
                        nc.vector.scalar_tensor_tensor(
                            attnf[b][:, h, qcs], at_ps[:], SA, rec[:],
                            ALU.mult, ALU.mult,
                        )
                        yield

                qg = [gen_qk_chain("q", wq8, qf, m) for m in range(NH)]
                kg = [gen_qk_chain("k", wk8, kf, m) for m in range(NH)]
                vg = gen_v()
                if loader is not None:
                    # loader FIRST and fast enough that every xn8k DMA is
                    # emitted before the first chain matmul that reads it
                    # (the dependency tracker is emission-order based)
                    _interleave((loader, 4), qg[0], kg[0])
                    _interleave((loader, 4), qg[1], kg[1])
                    _interleave(loader)
                else:
                    _interleave(qg[0], kg[0])
                    _interleave(qg[1], kg[1])
                _interleave(qg[2], kg[2], (vg, 2))
                _interleave(qg[3], kg[3], (vg, 2))
                _interleave(vg)
                es_vp.close()
                es_qp.close()
                if b == 0:
                    for hp in range(2):
                        wt = ow_sp.tile([P, 2, NDT, P], F8, name=f"wo_sb_{hp}")
                        nc.sync.dma_start(out=wt[:], in_=wo8[hp])
                        wo_sb.append(wt)
                # PSUM: qk 2 + den 2 + at 2 + lg 2 = 8 banks
                es_ap = ExitStack()
                att_pp = es_ap.enter_context(
                    tc.tile_pool(name=f"att_p{b}", bufs=1, space="PSUM")
                )
                _interleave(gen_attn_head(att_pp, 0), gen_attn_head(att_pp, 1))
                _interleave(gen_attn_head(att_pp, 2), gen_attn_head(att_pp, 3))
                es_ap.close()

        # ============ o-projection + AR per batch ============
        def fire_ar(b, k):
            rows = slice(k * (D // 2), (k + 1) * (D // 2))
            nc.gpsimd.collective_compute(
                "AllReduce",
                ALU.add,
                replica_groups=[list(range(N_CORES))],
                ins=[cc_in[b][rows, :]],
                outs=[cc_out[b][k][:, :]],
            )

        def run_o_proj(b):
            """o-projection partials + x/8 residual; the eviction stream is
            split DVE / (ACT copy + Pool add) per chunk so no single engine
            paces the PE, and the AR triggers fire as early as possible."""
            with ExitStack() as es_o:
                o_sp = es_o.enter_context(
                    tc.tile_pool(name=f"o_s{b}", bufs=1, side="right")
                )
                o_pp = es_o.enter_context(
                    tc.tile_pool(name=f"o_p{b}", bufs=1, space="PSUM",
                                 side="right")
                )
                for dt in range(NDT):
                    for ch in range(NCH):
                        cs = slice(ch * 512, (ch + 1) * 512)
                        gcs = slice(b * T + ch * 512, b * T + (ch + 1) * 512)
                        xt = o_sp.tile([P, 512], F16, tag="xres", bufs=6,
                                       name=f"xr_{b}_{dt}_{ch}")
                        nc.sync.dma_start(
                            out=xt[:], in_=xs16[dt * P:(dt + 1) * P, gcs]
                        )
                        # bufs=3: banks 5-7, disjoint from the next batch's
                        # qk (0-2) and v (3-4) pools — no cross-phase WAR
                        ps = o_pp.tile([P, 512], F32, tag="o", bufs=3,
                                       name=f"pso_{b}_{dt}_{ch}")
                        for hp in range(2):
                            nc.tensor.matmul(
                                ps[:], wo_sb[hp][:, :, dt, :],
                                attnf[b][:, 2 * hp:2 * hp + 2, cs],
                                start=(hp == 0), stop=(hp == 1), perf_mode=DR,
                            )
                        osb = o_sp.tile([P, 512], F16, tag="osb", bufs=4,
                                        name=f"osb_{b}_{dt}_{ch}")
                        nc.vector.scalar_tensor_tensor(
                            osb[:], ps[:], IO, xt[:], ALU.mult, ALU.add
                        )
                        nc.sync.dma_start(
                            out=cc_in[b][dt * P:(dt + 1) * P, cs], in_=osb[:]
                        )
                    if dt == NDT // 2 - 1 or dt == NDT - 1:
                        k = 0 if dt < NDT // 2 else 1
                        if b == 1:
                            # pinned past the xmid-b0 loads (2.05) so the
                            # Pool queue never blocks them on o-b1's writes
                            with tc.tile_wait_until(2.1):
                                fire_ar(b, k)
                        else:
                            fire_ar(b, k)

        # ---------- phases A(0), O(0), A(1), O(1) ----------
        ld = gen_loader()
        run_batch_attn(0, loader=ld)
        run_o_proj(0)
        run_batch_attn(1)
        run_o_proj(1)

        es_o_w.close()
        es_xn.close()

        # ============ MLP phases (pinned after the ARs) ============
        def load_xmid(b, pool, xdst):
            # issued from the Pool DGE (only the AR triggers live there) so
            # these AR-gated loads never block weight/eviction DMA traffic
            for kp in range(NKP):
                xk = pool.tile([P, T], F16, name=f"xm{b}_{kp}")
                nc.gpsimd.dma_start(out=xk[:], in_=xmid_rows(b, kp))
                xdst.append(xk)

        def mlp_ffup(b, xmh, w_sp, h_sp, ff_pp, ms_pp, hsb):
            """norm2 (fp8 squares + DR ones-matmul) interleaved with the
            ff/up chains: ms groups are emitted between chains so the PE
            reaches them only after their squares exist, and bcast2 is
            ready before the first eviction's WAR deadline (ps bufs=3)."""
            ms_ps = ms_pp.tile([P, T], F32, tag="ms", bufs=1, name=f"ms_{b}")
            sq8 = []

            def emit_sq(j):  # squares for kp pair j -> fp8 pair tile
                sq = w_sp.tile([P, 2, T], F8, tag="sq", bufs=NKP // 2,
                               name=f"sq_{b}_{j}")
                for e in range(2):
                    nc.scalar.activation(sq[:, e, :], xmh[2 * j + e][:],
                                         AF.Square)
                sq8.append(sq)

            def emit_ms_group(g):  # 4 kp-pairs of ms accumulation
                for j in range(4 * g, 4 * g + 4):
                    for ch in range(NCH):
                        cs = slice(ch * 512, (ch + 1) * 512)
                        nc.tensor.matmul(
                            ms_ps[:, cs], ones8[:], sq8[j][:, :, cs],
                            start=(j == 0), stop=(j == KP8 - 1),
                            perf_mode=DR,
                        )

            def finish_norm2():
                lnt = w_sp.tile([P, T], F32, tag="lnt", bufs=1,
                                name=f"lnt_{b}")
                nc.scalar.activation(lnt[:], ms_ps[:], AF.Ln, bias=eps_sb[:],
                                     scale=1.0 / D)
                nc.scalar.activation(bcast2[b][:], lnt[:], AF.Exp, scale=-0.5)

            ffs = [None] * NFT

            def emit_chain_mm(m, which, wsrc):
                wt = w_sp.tile([P, NKP, P], F16, tag="wffu", bufs=3,
                               name=f"w{which}_{b}_{m}")
                nc.sync.dma_start(out=wt[:], in_=wsrc[m])
                pss = []
                for ch in range(NCH):
                    cs = slice(ch * 512, (ch + 1) * 512)
                    ps = ff_pp.tile([P, 512], F32, tag=f"ps_{which}", bufs=3,
                                    name=f"ps{which}_{b}_{m}_{ch}")
                    for kp in range(NKP):
                        nc.tensor.matmul(
                            ps[:], wt[:, kp, :], xmh[kp][:, cs],
                            start=(kp == 0), stop=(kp == NKP - 1),
                        )
                    pss.append(ps)
                return pss

            def emit_evict(m, which, pss):
                # reads bcast2 — must be emitted AFTER finish_norm2 so the
                # emission-order dependency tracker sees the write
                for ch in range(NCH):
                    cs = slice(ch * 512, (ch + 1) * 512)
                    nt = w_sp.tile([P, 512], F16, tag=f"nrm_{which}", bufs=3,
                                   name=f"nt{which}_{b}_{m}_{ch}")
                    nc.vector.scalar_tensor_tensor(
                        nt[:], pss[ch][:], 1.0, bcast2[b][:, cs],
                        ALU.mult, ALU.mult,
                    )
                    if which == "f":
                        nc.scalar.activation(ffs[m][:, cs], nt[:], AF.Silu)
                    else:
                        nc.vector.tensor_mul(hsb[m][:, cs], nt[:],
                                             ffs[m][:, cs])

            def emit_chain(m, which, wsrc):
                emit_evict(m, which, emit_chain_mm(m, which, wsrc))

            for j in range(KP8):
                emit_sq(j)
            for m in range(NFT):
                ffs[m] = w_sp.tile([P, T], F16, tag="ffs", bufs=3,
                                   name=f"ff_{b}_{m}")
                hsb.append(h_sp.tile([P, T], F16, tag=f"h{m}",
                                     name=f"h_{b}_{m}"))
            psf0 = emit_chain_mm(0, "f", wf_t)
            emit_ms_group(0)
            psu0 = emit_chain_mm(0, "u", wu_t)
            emit_ms_group(1)
            emit_ms_group(2)
            emit_ms_group(3)
            finish_norm2()
            emit_evict(0, "f", psf0)
            emit_evict(0, "u", psu0)
            for m in range(1, NFT):
                emit_chain(m, "f", wf_t)
                emit_chain(m, "u", wu_t)

        def emit_wout(b, xmh, hsb, wo2_sp, wo2_pp):
            for dt in range(NDT):
                wt = wo2_sp.tile([P, NFT, P], F16, tag="wot", bufs=2,
                                 name=f"wot_{b}_{dt}")
                nc.sync.dma_start(out=wt[:], in_=wout_t[dt])
                for ch in range(NCH):
                    cs = slice(ch * 512, (ch + 1) * 512)
                    ps = wo2_pp.tile([P, 512], F32, tag="o2", bufs=2,
                                     name=f"pso2_{b}_{dt}_{ch}")
                    for mm in range(NFT):
                        nc.tensor.matmul(
                            ps[:], wt[:, mm, :], hsb[mm][:, cs],
                            start=(mm == 0), stop=(mm == NFT - 1),
                        )
                    ysb = wo2_sp.tile([P, 512], F32, tag="ysb", bufs=2,
                                      name=f"ysb_{b}_{dt}_{ch}")
                    nc.vector.scalar_tensor_tensor(
                        ysb[:], xmh[dt][:, cs], 1.0 / N_CORES, ps[:],
                        ALU.mult, ALU.add,
                    )
                    nc.sync.dma_start(
                        out=y[dt * P:(dt + 1) * P,
                              b * T + ch * 512:b * T + (ch + 1) * 512],
                        in_=ysb[:],
                    )

        # P7: x_mid b0 loads — Pool queue, after the b0 triggers (unpinned)
        # and before the b1 triggers (2.1)
        es_x0 = ExitStack()
        x0_sp = es_x0.enter_context(tc.tile_pool(name="x0_sp", bufs=1))
        xmh0 = []
        with tc.tile_wait_until(2.05):
            load_xmid(0, x0_sp, xmh0)

        # P8: norm2 b0 + ffup b0 (PSUM: ms 2 + ps_f 3 + ps_u 3 = 8 banks)
        hsb0 = []
        with tc.tile_wait_until(2.2):
            with ExitStack() as es_f0:
                f0w_sp = es_f0.enter_context(
                    tc.tile_pool(name="f0w_sp", bufs=1, side="right")
                )
                ff0_pp = es_f0.enter_context(
                    tc.tile_pool(name="f0_p", bufs=1, space="PSUM")
                )
                ms0_pp = es_f0.enter_context(
                    tc.tile_pool(name="ms0_p", bufs=1, space="PSUM",
                                 side="right")
                )
                mlp_ffup(0, xmh0, f0w_sp, x0_sp, ff0_pp, ms0_pp, hsb0)

        # P9: wout b0
        with tc.tile_wait_until(2.6):
            with ExitStack() as es_w0:
                w0_sp = es_w0.enter_context(
                    tc.tile_pool(name="w0_sp", bufs=1, side="right")
                )
                w0_pp = es_w0.enter_context(
                    tc.tile_pool(name="w0_p", bufs=1, space="PSUM")
                )
                emit_wout(0, xmh0, hsb0, w0_sp, w0_pp)

        # P9.5: x_mid b1 loads (SP, after wout-b0 weights)
        es_x1 = ExitStack()
        x1_sp = es_x1.enter_context(tc.tile_pool(name="x1_sp", bufs=1,
                                                 side="right"))
        xmh1 = []
        with tc.tile_wait_until(2.65):
            load_xmid(1, x1_sp, xmh1)
        es_x0.close()

        # P10: norm2 b1 + ffup b1
        hsb1 = []
        with tc.tile_wait_until(2.7):
            with ExitStack() as es_f1:
                f1w_sp = es_f1.enter_context(
                    tc.tile_pool(name="f1w_sp", bufs=1)
                )
                ff1_pp = es_f1.enter_context(
                    tc.tile_pool(name="f1_p", bufs=1, space="PSUM")
                )
                ms1_pp = es_f1.enter_context(
                    tc.tile_pool(name="ms1_p", bufs=1, space="PSUM",
                                 side="right")
                )
                mlp_ffup(1, xmh1, f1w_sp, x1_sp, ff1_pp, ms1_pp, hsb1)

        # P11: wout b1
        with tc.tile_wait_until(3.0):
            with ExitStack() as es_w1:
                w1_sp = es_w1.enter_context(
                    tc.tile_pool(name="w1_sp", bufs=1)
                )
                w1_pp = es_w1.enter_context(
                    tc.tile_pool(name="w1_p", bufs=1, space="PSUM")
                )
                emit_wout(1, xmh1, hsb1, w1_sp, w1_pp)
        es_x1.close()
        es_glob.close()


_NC_CACHE = {}


def _get_nc():
    if "nc" not in _NC_CACHE:
        _NC_CACHE["nc"] = _build()
    return _NC_CACHE["nc"]


def _host_prep(x, sin, cos, attn_norm_w, ff_norm_w, wq, wk, wv, wo, w_ff, w_up, w_out):
    f16 = np.float16
    f8 = ml_dtypes.float8_e4m3
    x2 = np.asarray(x, np.float32).reshape(M, D)
    xT = np.ascontiguousarray(x2.T)  # [D, M]

    # host norm1: per-token rms scale folded into a pre-normalized xn
    rs1 = 1.0 / np.sqrt((x2 * x2).mean(-1) + EPS)  # [M]
    xn = xT * rs1[None, :]
    # fp8 pair-packed [kp, p, e, t]: contraction k = kp*256 + e*128 + p
    xn8 = np.ascontiguousarray(
        (xn * SX).astype(f8).reshape(KP8, 2, P, M).transpose(0, 2, 1, 3)
    )

    sinT = np.asarray(sin, np.float32).reshape(M, HALF).T
    cosT = np.asarray(cos, np.float32).reshape(M, HALF).T
    cc = np.concatenate([cosT, cosT], axis=0)
    ss = np.concatenate([sinT, sinT], axis=0)
    css = np.stack([cc, ss]).astype(f16)

    anw = np.asarray(attn_norm_w, np.float32)[:, None]
    fnw = np.asarray(ff_norm_w, np.float32)[:, None]
    wqn = (anw * np.asarray(wq, np.float32)) * (H ** -0.5)
    wkn = anw * np.asarray(wk, np.float32)
    wvn = anw * np.asarray(wv, np.float32)
    wfn = fnw * np.asarray(w_ff, np.float32)
    wun = fnw * np.asarray(w_up, np.float32)
    wo_f = np.asarray(wo, np.float32)
    w_out_f = np.asarray(w_out, np.float32)
    # x/8 residual: every core adds this in the o-proj eviction, so the
    # AllReduce over 8 cores reconstructs x exactly once.
    xs16 = (xT * (1.0 / N_CORES)).astype(f16)

    def pack_qk(w):  # [D, QC] -> [NH, P, KP8, 2, P] fp8, scaled
        return np.ascontiguousarray(
            (w * SW).astype(f8).reshape(KP8, 2, P, NH, P).transpose(3, 2, 0, 1, 4)
        )

    def mtile(w):
        # [K, F] -> [F/P, P, K/P, P] with [m, p, kp, j] = w[kp*P+p, m*P+j]
        K, F = w.shape
        return np.ascontiguousarray(
            w.reshape(K // P, P, F // P, P).transpose(2, 1, 0, 3)
        )

    in_maps = []
    for c in range(N_CORES):
        qs = slice(c * QC, (c + 1) * QC)
        fs = slice(c * FC, (c + 1) * FC)
        wv8 = np.ascontiguousarray(
            (wvn[:, qs] * SW).astype(f8).reshape(KP8, 2, P, QC).transpose(2, 0, 1, 3)
        )
        wo8 = np.ascontiguousarray(
            (wo_f[qs, :] * SW).astype(f8).reshape(2, 2, P, NDT, P)
            .transpose(0, 2, 1, 3, 4)
        )
        in_maps.append(
            {
                "xn8": xn8,
                "xs16": xs16,
                "css": css,
                "wq8": pack_qk(wqn[:, qs]),
                "wk8": pack_qk(wkn[:, qs]),
                "wv8": wv8,
                "wo8": wo8,
                "wf_t": mtile(wfn[:, fs]).astype(f16),
                "wu_t": mtile(wun[:, fs]).astype(f16),
                "wout_t": mtile(w_out_f[fs, :]).astype(f16),
            }
        )
    return in_maps


def kernel(**inputs) -> np.ndarray:
    nc = _get_nc()
    in_maps = _host_prep(**inputs)
    res = run_bass_kernel_spmd(
        nc, in_maps, core_ids=list(range(N_CORES)), trace=False
    )
    acc = res.results[0]["y"].astype(np.float64)
    for c in range(1, N_CORES):
        acc += res.results[c]["y"]
    return np.ascontiguousarray(acc.T).astype(np.float32).reshape(B, T, D)


# revision 23
# speedup vs baseline: 1.0741x; 1.0492x over previous
"""TP-8 Trainium2 Bass kernel for a LLaDA/Llama transformer block.

Design (v4 — per-batch-serial schedule, queue-disciplined):
 - norm1 runs on the host: the device receives pre-normalized xn in
   fp8e4m3 (pair-packed for DoubleRow) and x.T/8 in fp16 (xs16).
 - q/k/v and o projections run fp8e4m3 with perf_mode=DoubleRow.
 - The residual joins INSIDE the o-projection eviction (osb = ps*IO +
   x/8, a single DVE stt), so the AllReduce output is x_mid directly
   and no post-AR assemble pass exists.
 - Per-batch-serial phases shorten the AR critical path: the whole b1
   attention pipeline covers AR-b0, and the b0 MLP covers AR-b1.
 - Queue discipline (the in-order engine queues are the whole game):
   Pool queue carries ONLY the 4 AR triggers; AR-gated x_mid loads sit
   on the SP queue pinned AFTER the MLP weight loads they'd otherwise
   block; norm2's square-sum matmuls are emitted in groups interleaved
   between ffup chains so the PE never head-of-line blocks on them.
 - norm2: squares in fp8 pair layout (ACT), sum via fp8-DR matmul with
   a ones stationary (2x fewer, 2x faster ms matmuls; quantization of
   x^2 averages out over D=4096).
 - Attention runs as head PAIRS with a one-step lg/exp software
   pipeline so den/at matmuls never wait on the Scalar exp latency.
 - MLP stays fp16 (fp8 there exceeds the 2e-2 error budget; measured:
   qkv fp8 8.4e-3 ok, ff/up fp8 4.6e-2 / wout fp8 2.4e-2 too large).

Sharding (per sharding_hint): tensor-parallel over 8 cores - q/k/v/ff
sharded on the output-feature axis (4 heads / 1536 ff dims per core),
wo/w_out sharded on the contraction axis; o-projection partials (+x/8)
AllReduced on device (fp16), final projection partials summed on host.
"""

from contextlib import ExitStack

import numpy as np
import ml_dtypes

import concourse.mybir as mybir
import concourse.tile as tile
from concourse import bacc
from concourse.bass_utils import run_bass_kernel_spmd

F32 = mybir.dt.float32
F16 = mybir.dt.float16
F8 = mybir.dt.float8e4
AF = mybir.ActivationFunctionType
ALU = mybir.AluOpType
DR = mybir.MatmulPerfMode.DoubleRow

N_CORES = 8
P = 128
B, T, D, FF = 2, 1024, 4096, 12288
M = B * T            # 2048 tokens
H = 128              # head dim
HALF = 64
QC = D // N_CORES    # 512 per-core q/k/v features (4 heads)
NH = QC // H         # 4 heads per core
FC = FF // N_CORES   # 1536 per-core ff features
NKP = D // P         # 32 K-tiles over D (fp16 granularity)
KP8 = NKP // 2       # 16 fp8 DoubleRow K-pair tiles
NFT = FC // P        # 12 M-tiles over per-core FF
NDT = D // P         # 32 D-tiles
NST = T // P         # 8 sequence tiles per batch
NCH = T // 512       # 2 column chunks per batch
EPS = 1e-05

SW = 16.0            # fp8 weight pre-scale
SX = 8.0             # fp8 xn pre-scale
SA = 32.0            # fp8 attnf pre-scale
IQK = 1.0 / (SW * SX)
IO = 1.0 / (SA * SW)


def _interleave(*items):
    """Drive generators to completion round-robin; an item may be a
    (generator, weight) tuple to take `weight` steps per round."""
    live = [[it[0], it[1]] if isinstance(it, tuple) else [it, 1] for it in items]
    while live:
        nxt = []
        for p in live:
            g, w = p
            alive = True
            for _ in range(w):
                try:
                    next(g)
                except StopIteration:
                    alive = False
                    break
            if alive:
                nxt.append(p)
        live = nxt


def _build():
    nc = bacc.Bacc("TRN2", target_bir_lowering=False, num_devices=N_CORES)

    xn8 = nc.declare_dram_parameter("xn8", [KP8, P, 2, M], F8, isOutput=False)
    xs16 = nc.declare_dram_parameter("xs16", [D, M], F16, isOutput=False)
    css = nc.declare_dram_parameter("css", [2, P, M], F16, isOutput=False)
    wq8 = nc.declare_dram_parameter("wq8", [NH, P, KP8, 2, P], F8, isOutput=False)
    wk8 = nc.declare_dram_parameter("wk8", [NH, P, KP8, 2, P], F8, isOutput=False)
    wv8 = nc.declare_dram_parameter("wv8", [P, KP8, 2, QC], F8, isOutput=False)
    wo8 = nc.declare_dram_parameter("wo8", [2, P, 2, NDT, P], F8, isOutput=False)
    wf_t = nc.declare_dram_parameter("wf_t", [NFT, P, NKP, P], F16, isOutput=False)
    wu_t = nc.declare_dram_parameter("wu_t", [NFT, P, NKP, P], F16, isOutput=False)
    wout_t = nc.declare_dram_parameter("wout_t", [NDT, P, NFT, P], F16, isOutput=False)
    y = nc.declare_dram_parameter("y", [D, M], F32, isOutput=True)

    with tile.TileContext(nc) as tc:
        _emit(nc, tc, xn8, xs16, css, wq8, wk8, wv8, wo8, wf_t, wu_t, wout_t, y)
    nc.compile()
    return nc


def _emit(nc, tc, xn8, xs16, css, wq8, wk8, wv8, wo8, wf_t, wu_t, wout_t, y):
    with ExitStack() as top:
        dram_pool = top.enter_context(tc.tile_pool(name="dram", bufs=1, space="DRAM"))
        const = top.enter_context(tc.tile_pool(name="const", bufs=1))

        cc_in = [dram_pool.tile([D, T], F16, name=f"cc_in_{b}") for b in range(B)]
        cc_out = [
            [
                dram_pool.tile([D // 2, T], F16, addr_space="Shared",
                               name=f"cc_out_{b}_{k}")
                for k in range(2)
            ]
            for b in range(B)
        ]

        def xmid_rows(b, kp):
            return cc_out[b][kp // (NKP // 2)][(kp % (NKP // 2)) * P:
                                               (kp % (NKP // 2) + 1) * P, :]

        ones_h = const.tile([P, P], F16)
        nc.vector.memset(ones_h[:], 1.0)
        ones8 = const.tile([P, 2, P], F8)
        nc.vector.memset(ones8[:], 1.0)
        eps_sb = const.tile([P, 1], F32)
        nc.vector.memset(eps_sb[:], EPS)

        # --- long-lived left-side tiles ---
        es_glob = ExitStack()
        glob = es_glob.enter_context(tc.tile_pool(name="glob", bufs=1))
        bcast2 = [glob.tile([P, T], F16, name=f"bcast2_{b}") for b in range(B)]
        attnf = [glob.tile([P, NH, T], F8, name=f"attnf_{b}") for b in range(B)]

        es_xn = ExitStack()
        xn_sp = es_xn.enter_context(tc.tile_pool(name="xn_sp", bufs=1))
        cc_sb = xn_sp.tile([P, M], F16, name="cc_sb")
        ss_sb = xn_sp.tile([P, M], F16, name="ss_sb")
        nc.sync.dma_start(out=cc_sb[:], in_=css[0])
        nc.sync.dma_start(out=ss_sb[:], in_=css[1])
        xn8k = []
        wv_sb = xn_sp.tile([P, KP8, 2, QC], F8, name="wv_sb")
        for kp in range(KP8):
            xn8k.append(xn_sp.tile([P, 2, M], F8, name=f"xn8_{kp}"))
        # first two xn tiles land before the first chain weights
        nc.sync.dma_start(out=xn8k[0][:], in_=xn8[0])
        nc.sync.dma_start(out=xn8k[1][:], in_=xn8[1])

        def gen_loader():
            """Streams the rest of the inputs between chain emissions so
            the first matmuls start ~10us in instead of ~45us."""
            for kp in range(2, KP8):
                nc.sync.dma_start(out=xn8k[kp][:], in_=xn8[kp])
                if kp == 9:
                    nc.sync.dma_start(out=wv_sb[:], in_=wv8[:])
                yield

        es_o_w = ExitStack()
        ow_sp = es_o_w.enter_context(tc.tile_pool(name="ow_sp", bufs=1))
        wo_sb = []

        # ============ attention superphase per batch ============
        def run_batch_attn(b, loader=None):
            with ExitStack() as es_a:
                qk_sp = es_a.enter_context(
                    tc.tile_pool(name=f"qk_s{b}", bufs=1, side="right")
                )
                bo_sp = es_a.enter_context(
                    tc.tile_pool(name=f"bo_s{b}", bufs=1, side="right")
                )
                es_qp = ExitStack()
                qk_pp = es_qp.enter_context(
                    tc.tile_pool(name=f"qk_p{b}", bufs=1, space="PSUM")
                )
                es_vp = ExitStack()
                v_pp = es_vp.enter_context(
                    tc.tile_pool(name=f"v_p{b}", bufs=1, space="PSUM",
                                 side="right")
                )

                qf = []
                kf = []
                v_sb = [None] * NST

                def gen_qk_chain(which, wsrc, dst, m):
                    wt = qk_sp.tile([P, KP8, 2, P], F8, tag="wqk", bufs=3,
                                    name=f"w{which}_{b}_{m}")
                    nc.sync.dma_start(out=wt[:], in_=wsrc[m])
                    out = bo_sp.tile([P, T], F16, name=f"{which}f_{b}_{m}")
                    for ch in range(NCH):
                        cs = slice(ch * 512, (ch + 1) * 512)
                        gcs = slice(b * T + ch * 512, b * T + (ch + 1) * 512)
                        ps = qk_pp.tile([P, 512], F32, tag="qk", bufs=3,
                                        name=f"ps{which}_{b}_{m}_{ch}")
                        for kp in range(KP8):
                            nc.tensor.matmul(
                                ps[:], wt[:, kp], xn8k[kp][:, :, gcs],
                                start=(kp == 0), stop=(kp == KP8 - 1),
                                perf_mode=DR,
                            )
                            if kp % 4 == 3:
                                yield
                        main = qk_sp.tile([P, 512], F16, tag="rmain", bufs=2,
                                          name=f"rm_{which}_{b}_{m}_{ch}")
                        nc.vector.scalar_tensor_tensor(
                            main[:], ps[:], IQK, cc_sb[:, gcs],
                            ALU.mult, ALU.mult,
                        )
                        rot = qk_sp.tile([P, 512], F16, tag="rrot", bufs=2,
                                         name=f"rr_{which}_{b}_{m}_{ch}")
                        nc.vector.scalar_tensor_tensor(
                            rot[:HALF], ps[HALF:], -IQK, ss_sb[:HALF, gcs],
                            ALU.mult, ALU.mult,
                        )
                        nc.vector.scalar_tensor_tensor(
                            rot[HALF:], ps[:HALF], IQK, ss_sb[HALF:, gcs],
                            ALU.mult, ALU.mult,
                        )
                        nc.vector.tensor_add(out[:, cs], main[:], rot[:])
                        yield
                    dst.append(out)

                def gen_v():
                    for r in range(NST // 2):
                        sts = (2 * r, 2 * r + 1)
                        psv = {}
                        for st in sts:
                            psv[st] = v_pp.tile([P, QC], F32, tag="vps",
                                                bufs=2, name=f"psv_{b}_{st}")
                        for kp in range(KP8):
                            for st in sts:
                                t0 = b * T + st * P
                                nc.tensor.matmul(
                                    psv[st][:], xn8k[kp][:, :, t0:t0 + P],
                                    wv_sb[:, kp],
                                    start=(kp == 0), stop=(kp == KP8 - 1),
                                    perf_mode=DR,
                                )
                            if kp % 4 == 3:
                                yield
                        for st in sts:
                            vt = bo_sp.tile([P, QC], F16, name=f"v_{b}_{st}")
                            nc.scalar.activation(vt[:], psv[st][:], AF.Copy,
                                                 scale=IQK)
                            v_sb[st] = vt
                        yield

                def gen_attn_head(att_pp, h):
                    """One-step lg/exp software pipeline; caller alternates
                    two heads so PE never waits on the exp latency."""
                    for qch in range(NCH):
                        qcs = slice(qch * 512, (qch + 1) * 512)
                        den_ps = att_pp.tile([P, 512], F32, tag="den", bufs=2,
                                             name=f"den_{b}_{h}_{qch}")
                        at_ps = att_pp.tile([P, 512], F32, tag="at", bufs=2,
                                            name=f"at_{b}_{h}_{qch}")

                        def emit_lgexp(st):
                            lg = att_pp.tile([P, 512], F32, tag="lg", bufs=2,
                                             name=f"lg_{b}_{h}_{qch}_{st}")
                            nc.tensor.matmul(
                                lg[:], kf[h][:, st * P:(st + 1) * P],
                                qf[h][:, qcs], start=True, stop=True,
                            )
                            pr = qk_sp.tile([P, 512], F16, tag="pr", bufs=4,
                                            name=f"pr_{b}_{h}_{qch}_{st}")
                            nc.scalar.activation(pr[:], lg[:], AF.Exp)
                            return pr

                        prs = [None] * NST
                        prs[0] = emit_lgexp(0)
                        yield
                        for st in range(NST):
                            if st + 1 < NST:
                                prs[st + 1] = emit_lgexp(st + 1)
                            yield
                            pr = prs[st]
                            nc.tensor.matmul(
                                den_ps[:], ones_h[:], pr[:],
                                start=(st == 0), stop=(st == NST - 1),
                            )
                            nc.tensor.matmul(
                                at_ps[:], v_sb[st][:, h * H:(h + 1) * H],
                                pr[:],
                                start=(st == 0), stop=(st == NST - 1),
                            )
                            yield
                        rec = qk_sp.tile([P, 512], F32, tag="rec", bufs=2,
                                         name=f"rec_{b}_{h}_{qch}")
                        nc.vector.reciprocal_approx_fast(rec[:], den_ps[:])
                        nc.vector.scalar_tensor_tensor(
                            attnf[b][:, h, qcs], at_ps[:], SA, rec[:],
                            ALU.mult, ALU.mult,
                        )
                        yield

                qg = [gen_qk_chain("q", wq8, qf, m) for m in range(NH)]
                kg = [gen_qk_chain("k", wk8, kf, m) for m in range(NH)]
                vg = gen_v()
                if loader is not None:
                    # loader FIRST and fast enough that every xn8k DMA is
                    # emitted before the first chain matmul that reads it
                    # (the dependency tracker is emission-order based)
                    _interleave((loader, 4), qg[0], kg[0])
                    _interleave((loader, 4), qg[1], kg[1])
                    _interleave(loader)
                else:
                    _interleave(qg[0], kg[0])
                    _interleave(qg[1], kg[1])
                _interleave(qg[2], kg[2], (vg, 2))
                _interleave(qg[3], kg[3], (vg, 2))
                _interleave(vg)
                es_vp.close()
                es_qp.close()
                if b == 0:
                    for hp in range(2):
                        wt = ow_sp.tile([P, 2, NDT, P], F8, name=f"wo_sb_{hp}")
                        nc.sync.dma_start(out=wt[:], in_=wo8[hp])
                        wo_sb.append(wt)
                # PSUM: qk 2 + den 2 + at 2 + lg 2 = 8 banks
                es_ap = ExitStack()
                att_pp = es_ap.enter_context(
                    tc.tile_pool(name=f"att_p{b}", bufs=1, space="PSUM")
                )
                _interleave(gen_attn_head(att_pp, 0), gen_attn_head(att_pp, 1))
                _interleave(gen_attn_head(att_pp, 2), gen_attn_head(att_pp, 3))
                es_ap.close()

        # ============ o-projection + AR per batch ============
        def fire_ar(b, k):
            rows = slice(k * (D // 2), (k + 1) * (D // 2))
            nc.gpsimd.collective_compute(
                "AllReduce",
                ALU.add,
                replica_groups=[list(range(N_CORES))],
                ins=[cc_in[b][rows, :]],
                outs=[cc_out[b][k][:, :]],
            )

        def run_o_proj(b):
            """o-projection partials + x/8 residual; the eviction stream is
            split DVE / (ACT copy + Pool add) per chunk so no single engine
            paces the PE, and the AR triggers fire as early as possible."""
            with ExitStack() as es_o:
                o_sp = es_o.enter_context(
                    tc.tile_pool(name=f"o_s{b}", bufs=1, side="right")
                )
                o_pp = es_o.enter_context(
                    tc.tile_pool(name=f"o_p{b}", bufs=1, space="PSUM",
                                 side="right")
                )
                for dt in range(NDT):
                    for ch in range(NCH):
                        cs = slice(ch * 512, (ch + 1) * 512)
                        gcs = slice(b * T + ch * 512, b * T + (ch + 1) * 512)
                        xt = o_sp.tile([P, 512], F16, tag="xres", bufs=6,
                                       name=f"xr_{b}_{dt}_{ch}")
                        nc.sync.dma_start(
                            out=xt[:], in_=xs16[dt * P:(dt + 1) * P, gcs]
                        )
                        # bufs=3: banks 5-7, disjoint from the next batch's
                        # qk (0-2) and v (3-4) pools — no cross-phase WAR
                        ps = o_pp.tile([P, 512], F32, tag="o", bufs=3,
                                       name=f"pso_{b}_{dt}_{ch}")
                        for hp in range(2):
                            nc.tensor.matmul(
                                ps[:], wo_sb[hp][:, :, dt, :],
                                attnf[b][:, 2 * hp:2 * hp + 2, cs],
                                start=(hp == 0), stop=(hp == 1), perf_mode=DR,
                            )
                        osb = o_sp.tile([P, 512], F16, tag="osb", bufs=4,
                                        name=f"osb_{b}_{dt}_{ch}")
                        if dt % 2 == 0:
                            nc.vector.scalar_tensor_tensor(
                                osb[:], ps[:], IO, xt[:], ALU.mult, ALU.add
                            )
                        else:
                            tmp = o_sp.tile([P, 512], F16, tag="otmp",
                                            bufs=4, name=f"ot_{b}_{dt}_{ch}")
                            nc.scalar.activation(tmp[:], ps[:], AF.Copy,
                                                 scale=IO)
                            nc.gpsimd.tensor_add(osb[:], tmp[:], xt[:])
                        nc.sync.dma_start(
                            out=cc_in[b][dt * P:(dt + 1) * P, cs], in_=osb[:]
                        )
                    if dt == NDT // 2 - 1 or dt == NDT - 1:
                        fire_ar(b, 0 if dt < NDT // 2 else 1)

        # ---------- phases A(0), O(0), A(1), O(1) ----------
        ld = gen_loader()
        run_batch_attn(0, loader=ld)
        run_o_proj(0)
        run_batch_attn(1)
        # pinned between the xmid-b0 loads (2.05) and the MLP (2.2) so
        # o-b1's eviction stream sorts cleanly on every engine queue
        with tc.tile_wait_until(2.1):
            run_o_proj(1)

        es_o_w.close()
        es_xn.close()

        # ============ MLP phases (pinned after the ARs) ============
        def load_xmid(b, pool, xdst):
            # issued from the Pool DGE (only the AR triggers live there) so
            # these AR-gated loads never block weight/eviction DMA traffic
            for kp in range(NKP):
                xk = pool.tile([P, T], F16, name=f"xm{b}_{kp}")
                nc.gpsimd.dma_start(out=xk[:], in_=xmid_rows(b, kp))
                xdst.append(xk)

        def mlp_ffup(b, xmh, w_sp, h_sp, ff_pp, ms_pp, hsb):
            """norm2 (fp8 squares + DR ones-matmul) interleaved with the
            ff/up chains: ms groups are emitted between chains so the PE
            reaches them only after their squares exist, and bcast2 is
            ready before the first eviction's WAR deadline (ps bufs=3)."""
            ms_ps = ms_pp.tile([P, T], F32, tag="ms", bufs=1, name=f"ms_{b}")
            sq8 = []

            def emit_sq(j):  # squares for kp pair j -> fp8 pair tile
                sq = w_sp.tile([P, 2, T], F8, tag="sq", bufs=NKP // 2,
                               name=f"sq_{b}_{j}")
                for e in range(2):
                    nc.scalar.activation(sq[:, e, :], xmh[2 * j + e][:],
                                         AF.Square)
                sq8.append(sq)

            def emit_ms_group(g):  # 4 kp-pairs of ms accumulation
                for j in range(4 * g, 4 * g + 4):
                    for ch in range(NCH):
                        cs = slice(ch * 512, (ch + 1) * 512)
                        nc.tensor.matmul(
                            ms_ps[:, cs], ones8[:], sq8[j][:, :, cs],
                            start=(j == 0), stop=(j == KP8 - 1),
                            perf_mode=DR,
                        )

            def finish_norm2():
                lnt = w_sp.tile([P, T], F32, tag="lnt", bufs=1,
                                name=f"lnt_{b}")
                nc.scalar.activation(lnt[:], ms_ps[:], AF.Ln, bias=eps_sb[:],
                                     scale=1.0 / D)
                nc.scalar.activation(bcast2[b][:], lnt[:], AF.Exp, scale=-0.5)

            ffs = [None] * NFT

            def emit_chain_mm(m, which, wsrc):
                wt = w_sp.tile([P, NKP, P], F16, tag="wffu", bufs=3,
                               name=f"w{which}_{b}_{m}")
                # Scalar DGE: the SP queue carries the previous wout's 64
                # y-writes, which would delay these for ~16us
                nc.scalar.dma_start(out=wt[:], in_=wsrc[m])
                pss = []
                for ch in range(NCH):
                    cs = slice(ch * 512, (ch + 1) * 512)
                    ps = ff_pp.tile([P, 512], F32, tag=f"ps_{which}", bufs=3,
                                    name=f"ps{which}_{b}_{m}_{ch}")
                    for kp in range(NKP):
                        nc.tensor.matmul(
                            ps[:], wt[:, kp, :], xmh[kp][:, cs],
                            start=(kp == 0), stop=(kp == NKP - 1),
                        )
                    pss.append(ps)
                return pss

            def emit_evict(m, which, pss):
                # reads bcast2 — must be emitted AFTER finish_norm2 so the
                # emission-order dependency tracker sees the write
                for ch in range(NCH):
                    cs = slice(ch * 512, (ch + 1) * 512)
                    nt = w_sp.tile([P, 512], F16, tag=f"nrm_{which}", bufs=3,
                                   name=f"nt{which}_{b}_{m}_{ch}")
                    nc.vector.scalar_tensor_tensor(
                        nt[:], pss[ch][:], 1.0, bcast2[b][:, cs],
                        ALU.mult, ALU.mult,
                    )
                    if which == "f":
                        nc.scalar.activation(ffs[m][:, cs], nt[:], AF.Silu)
                    else:
                        nc.vector.tensor_mul(hsb[m][:, cs], nt[:],
                                             ffs[m][:, cs])

            def emit_chain(m, which, wsrc):
                emit_evict(m, which, emit_chain_mm(m, which, wsrc))

            for j in range(KP8):
                emit_sq(j)
            for m in range(NFT):
                ffs[m] = w_sp.tile([P, T], F16, tag="ffs", bufs=3,
                                   name=f"ff_{b}_{m}")
                hsb.append(h_sp.tile([P, T], F16, tag=f"h{m}",
                                     name=f"h_{b}_{m}"))
            psf0 = emit_chain_mm(0, "f", wf_t)
            emit_ms_group(0)
            psu0 = emit_chain_mm(0, "u", wu_t)
            emit_ms_group(1)
            emit_ms_group(2)
            emit_ms_group(3)
            finish_norm2()
            emit_evict(0, "f", psf0)
            emit_evict(0, "u", psu0)
            for m in range(1, NFT):
                emit_chain(m, "f", wf_t)
                emit_chain(m, "u", wu_t)

        def emit_wout(b, xmh, hsb, wo2_sp, wo2_pp):
            for dt in range(NDT):
                wt = wo2_sp.tile([P, NFT, P], F16, tag="wot", bufs=2,
                                 name=f"wot_{b}_{dt}")
                nc.sync.dma_start(out=wt[:], in_=wout_t[dt])
                for ch in range(NCH):
                    cs = slice(ch * 512, (ch + 1) * 512)
                    ps = wo2_pp.tile([P, 512], F32, tag="o2", bufs=2,
                                     name=f"pso2_{b}_{dt}_{ch}")
                    for mm in range(NFT):
                        nc.tensor.matmul(
                            ps[:], wt[:, mm, :], hsb[mm][:, cs],
                            start=(mm == 0), stop=(mm == NFT - 1),
                        )
                    ysb = wo2_sp.tile([P, 512], F32, tag="ysb", bufs=2,
                                      name=f"ysb_{b}_{dt}_{ch}")
                    nc.vector.scalar_tensor_tensor(
                        ysb[:], xmh[dt][:, cs], 1.0 / N_CORES, ps[:],
                        ALU.mult, ALU.add,
                    )
                    nc.sync.dma_start(
                        out=y[dt * P:(dt + 1) * P,
                              b * T + ch * 512:b * T + (ch + 1) * 512],
                        in_=ysb[:],
                    )

        # P7: x_mid b0 loads — Pool queue, after the b0 triggers (unpinned)
        # and before the b1 triggers (2.1)
        es_x0 = ExitStack()
        x0_sp = es_x0.enter_context(tc.tile_pool(name="x0_sp", bufs=1))
        xmh0 = []
        with tc.tile_wait_until(2.05):
            load_xmid(0, x0_sp, xmh0)

        # P8: norm2 b0 + ffup b0 (PSUM: ms 2 + ps_f 3 + ps_u 3 = 8 banks)
        hsb0 = []
        with tc.tile_wait_until(2.2):
            with ExitStack() as es_f0:
                f0w_sp = es_f0.enter_context(
                    tc.tile_pool(name="f0w_sp", bufs=1, side="right")
                )
                ff0_pp = es_f0.enter_context(
                    tc.tile_pool(name="f0_p", bufs=1, space="PSUM")
                )
                ms0_pp = es_f0.enter_context(
                    tc.tile_pool(name="ms0_p", bufs=1, space="PSUM",
                                 side="right")
                )
                mlp_ffup(0, xmh0, f0w_sp, x0_sp, ff0_pp, ms0_pp, hsb0)

        # P9: wout b0
        with tc.tile_wait_until(2.6):
            with ExitStack() as es_w0:
                w0_sp = es_w0.enter_context(
                    tc.tile_pool(name="w0_sp", bufs=1, side="right")
                )
                w0_pp = es_w0.enter_context(
                    tc.tile_pool(name="w0_p", bufs=1, space="PSUM")
                )
                emit_wout(0, xmh0, hsb0, w0_sp, w0_pp)

        # P9.5: x_mid b1 loads (SP, after wout-b0 weights)
        es_x1 = ExitStack()
        x1_sp = es_x1.enter_context(tc.tile_pool(name="x1_sp", bufs=1,
                                                 side="right"))
        xmh1 = []
        with tc.tile_wait_until(2.65):
            load_xmid(1, x1_sp, xmh1)
        es_x0.close()

        # P10: norm2 b1 + ffup b1
        hsb1 = []
        with tc.tile_wait_until(2.7):
            with ExitStack() as es_f1:
                f1w_sp = es_f1.enter_context(
                    tc.tile_pool(name="f1w_sp", bufs=1)
                )
                ff1_pp = es_f1.enter_context(
                    tc.tile_pool(name="f1_p", bufs=1, space="PSUM")
                )
                ms1_pp = es_f1.enter_context(
                    tc.tile_pool(name="ms1_p", bufs=1, space="PSUM",
                                 side="right")
                )
                mlp_ffup(1, xmh1, f1w_sp, x1_sp, ff1_pp, ms1_pp, hsb1)

        # P11: wout b1
        with tc.tile_wait_until(3.0):
            with ExitStack() as es_w1:
                w1_sp = es_w1.enter_context(
                    tc.tile_pool(name="w1_sp", bufs=1)
                )
                w1_pp = es_w1.enter_context(
                    tc.tile_pool(name="w1_p", bufs=1, space="PSUM")
                )
                emit_wout(1, xmh1, hsb1, w1_sp, w1_pp)
        es_x1.close()
        es_glob.close()


_NC_CACHE = {}


def _get_nc():
    if "nc" not in _NC_CACHE:
        _NC_CACHE["nc"] = _build()
    return _NC_CACHE["nc"]


def _host_prep(x, sin, cos, attn_norm_w, ff_norm_w, wq, wk, wv, wo, w_ff, w_up, w_out):
    f16 = np.float16
    f8 = ml_dtypes.float8_e4m3
    x2 = np.asarray(x, np.float32).reshape(M, D)
    xT = np.ascontiguousarray(x2.T)  # [D, M]

    # host norm1: per-token rms scale folded into a pre-normalized xn
    rs1 = 1.0 / np.sqrt((x2 * x2).mean(-1) + EPS)  # [M]
    xn = xT * rs1[None, :]
    # fp8 pair-packed [kp, p, e, t]: contraction k = kp*256 + e*128 + p
    xn8 = np.ascontiguousarray(
        (xn * SX).astype(f8).reshape(KP8, 2, P, M).transpose(0, 2, 1, 3)
    )

    sinT = np.asarray(sin, np.float32).reshape(M, HALF).T
    cosT = np.asarray(cos, np.float32).reshape(M, HALF).T
    cc = np.concatenate([cosT, cosT], axis=0)
    ss = np.concatenate([sinT, sinT], axis=0)
    css = np.stack([cc, ss]).astype(f16)

    anw = np.asarray(attn_norm_w, np.float32)[:, None]
    fnw = np.asarray(ff_norm_w, np.float32)[:, None]
    wqn = (anw * np.asarray(wq, np.float32)) * (H ** -0.5)
    wkn = anw * np.asarray(wk, np.float32)
    wvn = anw * np.asarray(wv, np.float32)
    wfn = fnw * np.asarray(w_ff, np.float32)
    wun = fnw * np.asarray(w_up, np.float32)
    wo_f = np.asarray(wo, np.float32)
    w_out_f = np.asarray(w_out, np.float32)
    # x/8 residual: every core adds this in the o-proj eviction, so the
    # AllReduce over 8 cores reconstructs x exactly once.
    xs16 = (xT * (1.0 / N_CORES)).astype(f16)

    def pack_qk(w):  # [D, QC] -> [NH, P, KP8, 2, P] fp8, scaled
        return np.ascontiguousarray(
            (w * SW).astype(f8).reshape(KP8, 2, P, NH, P).transpose(3, 2, 0, 1, 4)
        )

    def mtile(w):
        # [K, F] -> [F/P, P, K/P, P] with [m, p, kp, j] = w[kp*P+p, m*P+j]
        K, F = w.shape
        return np.ascontiguousarray(
            w.reshape(K // P, P, F // P, P).transpose(2, 1, 0, 3)
        )

    in_maps = []
    for c in range(N_CORES):
        qs = slice(c * QC, (c + 1) * QC)
        fs = slice(c * FC, (c + 1) * FC)
        wv8 = np.ascontiguousarray(
            (wvn[:, qs] * SW).astype(f8).reshape(KP8, 2, P, QC).transpose(2, 0, 1, 3)
        )
        wo8 = np.ascontiguousarray(
            (wo_f[qs, :] * SW).astype(f8).reshape(2, 2, P, NDT, P)
            .transpose(0, 2, 1, 3, 4)
        )
        in_maps.append(
            {
                "xn8": xn8,
                "xs16": xs16,
                "css": css,
                "wq8": pack_qk(wqn[:, qs]),
                "wk8": pack_qk(wkn[:, qs]),
                "wv8": wv8,
                "wo8": wo8,
                "wf_t": mtile(wfn[:, fs]).astype(f16),
                "wu_t": mtile(wun[:, fs]).astype(f16),
                "wout_t": mtile(w_out_f[fs, :]).astype(f16),
            }
        )
    return in_maps


def kernel(**inputs) -> np.ndarray:
    nc = _get_nc()
    in_maps = _host_prep(**inputs)
    res = run_bass_kernel_spmd(
        nc, in_maps, core_ids=list(range(N_CORES)), trace=False
    )
    acc = res.results[0]["y"].astype(np.float64)
    for c in range(1, N_CORES):
        acc += res.results[c]["y"]
    return np.ascontiguousarray(acc.T).astype(np.float32).reshape(B, T, D)
